# revision 19
# baseline (speedup 1.0000x reference)
"""Causal self-attention with interleaved RoPE, tensor-parallel over heads on 8 NeuronCores.

Strategy (per core c, heads hA=2c, hB=2c+1), v2:
  - All on-chip tensors "transposed": feature dim on partitions, tokens on free dim.
  - bf16 for DMA-heavy tensors (x, qkv_w, out_w, V path, E, output partials);
    q/k kept f32r for score precision. Host sums partials in f32.
  - DMA batching: one big DMA per x strip / weight tensor / const blob / output
    block, all issued from the SP queue (HWDGE generation is a serial ~625ns
    resource and each DMA pays ~900ns sem propagation).
  - QKV projection per 512-token chunk: psum[dcol, tok] = qkv_wT.T @ x_strip.
    RoPE in-transposed layout: q_rot = q*cosT + swap(q*sinTt), swap = DVE
    stream_shuffle, sin table sign-folded/pair-reindexed on host.
  - Per-chunk q2c/k2c/v2c tiles; attention for query block j issues right after
    chunk j's QKV, so QKV(jc+1) overlaps attention(j=jc) on the other engines.
  - Scores transposed: S^T[tk,tq] = K^T.T @ Q^T per head, 2 heads packed via PE
    row tiling. exp on ACT (scale 1/8 + key-mask bias folded in); causal diag
    via memset-zero + one shared 128x128 tri mask; ACT does nothing but exp.
  - AV: yp[{d|r}, tq] += [V_h | ones].T @ E^T; ones cols in a persistent
    per-batch vsb tile (memset once) make rows 64:128 the softmax denominator.
  - Attention t-loop software-pipelined: AV(t) issued after S(t+1)/exp(t+1).
  - Out-proj partials out^T[c_out, tq] = owT.T @ y2c, staged via DVE/Pool copies
    (never ACT) into one [128, 4096] bf16 tile, one store DMA per (b, j).
  - Host: gathers 8 bf16 partial outputs, sums in f32, applies query mask/bias.
"""

import numpy as np

B, T, C = 2, 2048, 1024
H, DH = 16, 64
NCORES = 8
CT = C // 128  # 8 contraction tiles
NTK = T // 128  # 16 key tiles
NJ = T // 512  # 4 query blocks
NEG = -1e30

_PROGRAM_CACHE = {}
LAST_RESULTS = None


def _build_program(has_qkv_bias=False):
    import concourse.mybir as mybir
    import concourse.tile as tile
    from concourse import bacc
    from contextlib import ExitStack

    F32 = mybir.dt.float32
    F32R = mybir.dt.float32r
    BF16 = mybir.dt.bfloat16
    EXP = mybir.ActivationFunctionType.Exp

    SWAP_MASK = [i ^ 1 for i in range(32)]
    nc = bacc.Bacc("TRN2", target_bir_lowering=False, debug=False)

    # ---- DRAM I/O ----
    xT_d = nc.dram_tensor("xT", (CT, 128, B, T), BF16, kind="ExternalInput")
    qkvwT_d = nc.dram_tensor("qkvwT", (3, CT, 128, 128), BF16, kind="ExternalInput")
    bqkv_d = nc.dram_tensor("bqkv", (128, 3), F32, kind="ExternalInput")
    owT_d = nc.dram_tensor("owT", (8, 128, 128), BF16, kind="ExternalInput")
    # f32 const blob: [cos(T) | sinTt(T) | expb(B*NTK)]
    CSW = 2 * T + B * NTK
    cs_d = nc.dram_tensor("cs", (128, CSW), F32, kind="ExternalInput")
    # bf16 const blob: [tri(128) | ident(128)]
    bfc_d = nc.dram_tensor("bfc", (128, 256), BF16, kind="ExternalInput")
    idr_d = nc.dram_tensor("idr", (128, 128), F32R, kind="ExternalInput")
    outp_d = nc.dram_tensor("outp", (8, 128, B, T), BF16, kind="ExternalOutput")

    with tile.TileContext(nc) as tc, ExitStack() as ctx:
        cpool = ctx.enter_context(tc.tile_pool(name="consts", bufs=1))
        xpool = ctx.enter_context(tc.tile_pool(name="xt", bufs=3))
        spool = ctx.enter_context(tc.tile_pool(name="seq", bufs=8))
        vpool = ctx.enter_context(tc.tile_pool(name="vsb", bufs=2))
        epool = ctx.enter_context(tc.tile_pool(name="eexp", bufs=6))
        opool = ctx.enter_context(tc.tile_pool(name="outs", bufs=2))
        tpool = ctx.enter_context(tc.tile_pool(name="tmp", bufs=2))
        rpool = ctx.enter_context(tc.tile_pool(name="rr", bufs=2))
        spsum = ctx.enter_context(tc.tile_pool(name="S", bufs=2, space="PSUM"))
        qpool = ctx.enter_context(tc.tile_pool(name="qp", bufs=2, space="PSUM"))
        ypool = ctx.enter_context(tc.tile_pool(name="yp", bufs=2, space="PSUM"))

        # ---- const + input loads, batched, issue order = need order ----
        qkvw = cpool.tile([128, 3 * CT * 128], BF16, name="qkvw", tag="qkvw")

        def qkvw_dma(s):
            dst = qkvw[:, 1024 * s : 1024 * (s + 1)].rearrange(
                "p (k c) -> p k c", k=CT
            )
            nc.sync.dma_start(dst, qkvwT_d[s].rearrange("k p c -> p k c"))

        qkvw_dma(0)

        def strip_tile(b, half):
            t = xpool.tile([128, CT * 1024], BF16, tag="strip", name=f"xs{b}{half}")
            return t, t.rearrange("p (k c) -> p k c", k=CT)

        def strip_dma(tr, b, half, tok=slice(0, 1024)):
            sl = slice(1024 * half + tok.start, 1024 * half + tok.stop)
            nc.sync.dma_start(
                tr[:, :, tok], xT_d[:, :, b, sl].rearrange("k p t -> p k t")
            )

        strips = {}
        # chunk-0 data first (all k, tokens 0:512), then weights, then the
        # RoPE tables for the first chunks, then everything else
        strips[(0, 0)], tr00 = strip_tile(0, 0)
        strip_dma(tr00, 0, 0, slice(0, 512))
        cs = cpool.tile([128, CSW], F32, name="cs", tag="cs")
        qkvw_dma(1)
        qkvw_dma(2)
        nc.sync.dma_start(cs[:, 0:1024], cs_d[:, 0:1024])
        nc.sync.dma_start(cs[:, T : T + 1024], cs_d[:, T : T + 1024])
        nc.sync.dma_start(cs[:, 2 * T : CSW], cs_d[:, 2 * T : CSW])
        strip_dma(tr00, 0, 0, slice(512, 1024))
        bfc = cpool.tile([128, 256], BF16, name="bfc", tag="bfc")
        nc.sync.dma_start(bfc[:], bfc_d[:, :])
        identR = cpool.tile([128, 128], F32R, name="identR", tag="identR")
        nc.sync.dma_start(identR[:], idr_d[:, :])
        if has_qkv_bias:
            bqkv = cpool.tile([128, 3], F32, name="bqkv", tag="bqkv")
            nc.sync.dma_start(bqkv[:], bqkv_d[:, :])
        strips[(0, 1)], tr01 = strip_tile(0, 1)
        strip_dma(tr01, 0, 1)
        nc.sync.dma_start(cs[:, 1024:T], cs_d[:, 1024:T])
        nc.sync.dma_start(cs[:, T + 1024 : 2 * T], cs_d[:, T + 1024 : 2 * T])
        owT = cpool.tile([128, 8 * 128], BF16, name="owT", tag="owT")
        nc.sync.dma_start(
            owT.rearrange("p (m c) -> p m c", m=8),
            owT_d.rearrange("m p c -> p m c"),
        )

        strips[(1, 0)], tr10 = strip_tile(1, 0)
        strip_dma(tr10, 1, 0)
        strips[(1, 1)], tr11 = strip_tile(1, 1)
        strip_dma(tr11, 1, 1)

        cosT = cs[:, 0:T]
        sinTt = cs[:, T : 2 * T]
        expb = cs[:, 2 * T : 2 * T + B * NTK]
        triC = bfc[:, 0:128]

        # dummy exp so the ACT table set loads during the initial DMA fill
        warm = cpool.tile([128, 1], F32, name="warm", tag="warm")
        nc.vector.memset(warm[:], 0.0)
        nc.scalar.activation(warm[:], warm[:], EXP)

        state = {}  # b -> (q2c, k2c, v2c, y2c, vsb)
        workq = []  # deferred out-projection closures, drip-fed everywhere

        def qkv_chunk(b, jc):
            q2c, k2c, v2c, y2c, vsb = state[b]
            strip = strips[(b, jc // 2)]
            strip_r = strip.rearrange("p (k c) -> p k c", k=CT)
            xoff = 512 * (jc % 2)
            sl = slice(512 * jc, 512 * (jc + 1))

            # ---- QKV projection + RoPE for this 512-token chunk ----
            q2c[jc] = spool.tile([128, 512], F32R, tag="q2c", name=f"q{b}_{jc}")
            k2c[jc] = spool.tile([128, 512], F32R, tag="k2c", name=f"k{b}_{jc}")
            v2c[jc] = spool.tile(
                [128, 512], F32R, tag="v2c", bufs=3, name=f"v{b}_{jc}"
            )
            dsts = [q2c[jc], k2c[jc], v2c[jc]]
            for s in range(3):
                if workq:
                    workq.pop(0)()
                ps = qpool.tile([128, 512], F32, tag="qp")
                for k in range(CT):
                    nc.tensor.matmul(
                        ps[:],
                        qkvw[:, (s * CT + k) * 128 : (s * CT + k + 1) * 128],
                        strip_r[:, k, xoff : xoff + 512],
                        start=(k == 0),
                        stop=(k == CT - 1),
                    )
                if has_qkv_bias:
                    nc.vector.tensor_scalar_add(ps[:], ps[:], bqkv[:, s : s + 1])
                if s == 2:
                    nc.vector.tensor_copy(v2c[jc][:], ps[:])
                else:
                    t1 = tpool.tile([128, 512], F32, tag="t1")
                    t2 = tpool.tile([128, 512], F32, tag="t2")
                    t2s = tpool.tile([128, 512], F32, tag="t2s")
                    nc.vector.tensor_mul(t1[:], ps[:], cosT[:, sl])
                    nc.vector.tensor_mul(t2[:], ps[:], sinTt[:, sl])
                    nc.vector.stream_shuffle(t2s[:], t2[:], SWAP_MASK)
                    nc.gpsimd.tensor_add(dsts[s][:], t1[:], t2s[:])
            # transpose this chunk's V tiles into the persistent vsb tile
            if workq:
                workq.pop(0)()
            vtg = qpool.tile([128, 512], F32R, tag="qp", name=f"vtg{b}_{jc}")
            for u in range(4):
                nc.tensor.transpose(
                    vtg[:, 128 * u : 128 * (u + 1)],
                    v2c[jc][:, 128 * u : 128 * (u + 1)],
                    identR[:],
                )
            for u in range(4):
                t = 4 * jc + u
                dst = vsb[:, 256 * t : 256 * (t + 1)].rearrange(
                    "p (h c) -> p h c", h=2
                )[:, :, 0:64]
                src = vtg[:, 128 * u : 128 * (u + 1)].rearrange(
                    "p (h c) -> p h c", h=2
                )
                nc.vector.tensor_copy(dst, src)

        def attn_block(b, j):
            # ---- attention for query block j (2 heads packed),
            # software-pipelined: AV(t) trails S/exp(t+1); earlier blocks'
            # out-projection closures (workq) are drip-fed one per key tile
            # so they never head-of-line-block the PE ----
            q2c, k2c, v2c, y2c, vsb = state[b]
            jsl = slice(512 * j, 512 * (j + 1))
            yp = [
                ypool.tile([128, 512], F32, tag="yp", name=f"yp{b}_{j}_{h}")
                for h in range(2)
            ]
            ntk_j = 4 * (j + 1)
            pend = None  # (E tile, t) awaiting AV

            def issue_av(E, t, last):
                m = t - 4 * j if t >= 4 * j else -1
                qo = 128 * m if m >= 1 else 0
                for h in range(2):
                    nc.tensor.matmul(
                        yp[h][:, qo:512],
                        vsb[:, 256 * t + 128 * h : 256 * t + 128 * (h + 1)],
                        E[:, 512 * h + qo : 512 * (h + 1)],
                        start=(t == 0),
                        stop=last,
                    )

            for t in range(ntk_j):
                tc_i, u = t // 4, t % 4
                m = t - 4 * j if t >= 4 * j else -1
                # diagonal tiles: queries below the diagonal are fully masked —
                # restrict scores/exp/AV to the live query range instead of
                # zeroing (the masked E region is simply never touched or read)
                qo = 128 * m if m >= 1 else 0
                S = spsum.tile([128, 1024], F32, tag="S")
                for h in range(2):
                    hsl = slice(64 * h, 64 * (h + 1))
                    nc.tensor.matmul(
                        S[:, 512 * h + qo : 512 * (h + 1)],
                        k2c[tc_i][hsl, 128 * u : 128 * (u + 1)],
                        q2c[j][hsl, qo:512],
                        start=True,
                        stop=True,
                        tile_position=(64 * h, 0),
                    )
                E = epool.tile([128, 1024], BF16, tag="E")
                ecol = b * NTK + t
                if m >= 1:
                    seg = E.rearrange("p (h c) -> p h c", h=2)[:, :, qo:512]
                    sseg = S.rearrange("p (h c) -> p h c", h=2)[:, :, qo:512]
                    nc.scalar.activation(
                        seg, sseg, EXP, bias=expb[:, ecol : ecol + 1], scale=0.125
                    )
                else:
                    nc.scalar.activation(
                        E[:], S[:], EXP, bias=expb[:, ecol : ecol + 1], scale=0.125
                    )
                if m >= 0:
                    # partial-diagonal 128-col block: lower-tri mask
                    for h in range(2):
                        dsl = slice(512 * h + 128 * m, 512 * h + 128 * (m + 1))
                        nc.gpsimd.tensor_mul(E[:, dsl], E[:, dsl], triC[:])
                if pend is not None:
                    issue_av(pend[0], pend[1], False)
                pend = (E, t)
                if workq:
                    workq.pop(0)()
            issue_av(pend[0], pend[1], True)

            # normalize: y / rowsum, bf16 chunk for the out-projection
            y2c[j] = spool.tile(
                [128, 512], BF16, tag="y2c", bufs=3, name=f"y{b}_{j}"
            )
            for h in range(2):
                rr = rpool.tile([64, 512], F32, tag="rr")
                nc.vector.reciprocal(rr[:], yp[h][64:128, :])
                nc.vector.tensor_mul(
                    y2c[j][64 * h : 64 * (h + 1), :], yp[h][0:64, :], rr[:]
                )

            # ---- output projection closures, returned for the caller to
            # interleave into the next block ----
            ot = opool.tile([128, 8 * 512], BF16, tag="ot", name=f"ot{b}_{j}")
            yj = y2c[j]

            def op_step(mt, b=b, j=j, ot=ot, yj=yj, jsl=jsl):
                op = qpool.tile([128, 512], F32, tag="qp", name=f"op{b}_{j}_{mt}")
                nc.tensor.matmul(
                    op[:],
                    owT[:, 128 * mt : 128 * (mt + 1)],
                    yj[:],
                    start=True,
                    stop=True,
                )
                osl = slice(512 * mt, 512 * (mt + 1))
                if mt % 2 == 1 and j in (0, 3):
                    nc.scalar.copy(ot[:, osl], op[:])
                else:
                    nc.vector.tensor_copy(ot[:, osl], op[:])

            def st_half(i, b=b, ot=ot, jsl=jsl):
                nc.sync.dma_start(
                    outp_d[4 * i : 4 * (i + 1), :, b, jsl].rearrange(
                        "m p q -> p m q"
                    ),
                    ot[:, 2048 * i : 2048 * (i + 1)].rearrange(
                        "p (m q) -> p m q", m=4
                    ),
                )

            workq.extend([lambda mt=mt: op_step(mt) for mt in range(4)])
            workq.append(lambda: st_half(0))
            workq.extend([lambda mt=mt: op_step(mt) for mt in range(4, 8)])
            workq.append(lambda: st_half(1))

        for b in range(B):
            vsb = vpool.tile([128, NTK * 256], BF16, tag="vsb", name=f"vsb{b}")
            state[b] = ({}, {}, {}, {}, vsb)
            # ones columns (softmax denominator rows), once per batch
            nc.gpsimd.memset(
                vsb.rearrange("p (t h c) -> p t h c", t=NTK, h=2)[:, :, :, 64:128],
                1.0,
            )
            # QKV runs one chunk ahead of attention so attention-phase PE gaps
            # (waiting on exp) are filled with the next chunk's projection
            qkv_chunk(b, 0)
            qkv_chunk(b, 1)
            attn_block(b, 0)
            qkv_chunk(b, 2)
            attn_block(b, 1)
            qkv_chunk(b, 3)
            attn_block(b, 2)
            attn_block(b, 3)
        for f in workq:  # final block's out-projection: the kernel tail
            f()

    nc.compile()
    return nc


def _host_inputs(x, attention_mask, qkv_w, qkv_b, out_w):
    """Build the device input tensors. Returns (shared dict, per-core list of dicts)."""
    import ml_dtypes

    BF = ml_dtypes.bfloat16
    x = np.ascontiguousarray(np.asarray(x, np.float32))
    qkv_w = np.asarray(qkv_w, np.float32)
    qkv_b = np.asarray(qkv_b, np.float32)
    out_w = np.asarray(out_w, np.float32)
    am = np.asarray(attention_mask)

    xT = np.ascontiguousarray(x.transpose(2, 0, 1).reshape(CT, 128, B, T)).astype(BF)

    # RoPE tables (match reference: interleaved rotate, concatenated freq table)
    inv_freq = 1.0 / (10000.0 ** (np.arange(0, DH, 2, dtype=np.float64) / DH))
    tt = np.arange(T, dtype=np.float64)
    freqs = np.outer(tt, inv_freq)  # [T, 32]
    emb = np.concatenate([freqs, freqs], axis=-1)  # [T, 64]
    cos = np.cos(emb).astype(np.float32).T  # [64, T]
    sin = np.sin(emb).astype(np.float32).T  # [64, T]
    sinTt64 = np.empty((DH, T), np.float32)
    sinTt64[0::2] = sin[1::2]  # sinTt[2i]   = +sin[2i+1]
    sinTt64[1::2] = -sin[0::2]  # sinTt[2i+1] = -sin[2i]
    cosT = np.tile(cos, (2, 1))  # [128, T]
    sinTt = np.tile(sinTt64, (2, 1))

    key_ok = am.astype(bool).reshape(B, NTK, 128)  # [b, t, p]
    expb = np.where(key_ok, 0.0, NEG).astype(np.float32)
    expb = expb.transpose(2, 0, 1).reshape(128, B * NTK)
    cs = np.ascontiguousarray(np.concatenate([cosT, sinTt, expb], axis=1))

    cc = np.arange(128)[None, :]
    pp = np.arange(128)[:, None]
    tri = (cc >= pp).astype(np.float32)
    ident = np.eye(128, dtype=np.float32)
    bfc = np.ascontiguousarray(np.concatenate([tri, ident], axis=1)).astype(BF)

    shared = dict(xT=xT, cs=cs, bfc=bfc, idr=ident)

    per_core = []
    for c in range(NCORES):
        r0 = 128 * c
        qkvwT = np.stack(
            [
                np.ascontiguousarray(
                    qkv_w[s * C + r0 : s * C + r0 + 128, :].T
                ).reshape(CT, 128, 128)
                for s in range(3)
            ]
        ).astype(BF)
        bqkv = np.stack(
            [qkv_b[s * C + r0 : s * C + r0 + 128] for s in range(3)], axis=1
        )  # [128, 3]
        ow_slice = out_w[:, r0 : r0 + 128]  # [1024, 128]
        owT = np.ascontiguousarray(
            ow_slice.reshape(8, 128, 128).transpose(0, 2, 1)
        ).astype(BF)
        per_core.append(
            dict(qkvwT=qkvwT, bqkv=np.ascontiguousarray(bqkv), owT=owT)
        )
    return shared, per_core


def kernel(x, attention_mask, qkv_w, qkv_b, out_w, out_b, _trace=False):
    global LAST_RESULTS
    from concourse.bass_utils import run_bass_kernel_spmd

    key = ("nc", bool(np.any(np.asarray(qkv_b))))
    if key not in _PROGRAM_CACHE:
        _PROGRAM_CACHE[key] = _build_program(has_qkv_bias=key[1])
    nc = _PROGRAM_CACHE[key]

    shared, per_core = _host_inputs(x, attention_mask, qkv_w, qkv_b, out_w)
    in_maps = [{**shared, **pc} for pc in per_core]

    res = run_bass_kernel_spmd(
        nc,
        in_maps,
        core_ids=list(range(NCORES)),
        trace=_trace,
        trace_cores=list(range(NCORES)) if _trace else None,
        stitch_traces=bool(_trace),
    )
    LAST_RESULTS = res

    acc = np.zeros((B, T, C), np.float64)
    for c in range(NCORES):
        part = np.asarray(res.results[c]["outp"], np.float32)  # [8, 128, B, T]
        acc += part.transpose(2, 3, 0, 1).reshape(B, T, C)

    qm = np.asarray(attention_mask).astype(bool)
    out = np.where(qm[..., None], acc, 0.0) + np.asarray(out_b, np.float64)[None, None]
    return out.astype(np.float32)
